# revision 46
# baseline (speedup 1.0000x reference)
"""MoE layer (8 experts, top-2) on 8 Trainium2 NeuronCores — paired-expert load-balanced expert parallelism.

Experts are paired (largest with smallest by token count); each pair is
hosted by two cores, each core computing half of both experts' tokens.
Capacity per core C = SA + SB with UNPADDED segment sizes (no
128-alignment): both layers run column-exact matmuls, so padding waste
vs 128-rounding (~3%) is eliminated.

Device kernel (per slot, whole-segment loops):
  - x and h for the whole segment resident in SBUF; W1 streamed ONCE
    per slot per iteration (m-outer loop) — 42MB/core/iter total DMA.
  - Layer 1: out[h-block, tok] with W1 128x128 stationary blocks,
    tokens moving in <=512-column pieces.
  - Layer 2 swapped: out[d-block, tok] = W2-block stationary, h tokens
    moving — cost scales with exact token count, not 128-tiles. Output
    y is [d, tok] transposed, bf16; host untransposes and combines.
  - Single W2 SBUF bank (64KB/partition), reloaded per slot behind the
    slot's layer-1 (k-progressive slices feed layer-2's k-outer loop).
  - PSUM: 2 banks layer-1 double-buffer + 4-6 banks for 4 concurrent
    db-group accumulators; drains alternate DVE/ScalarE two-wide into a
    shared staging tile, one store descriptor per (db-group, piece).
"""

import numpy as np
import ml_dtypes

try:
    import concourse.bass as bass
except ImportError:
    import sys

    sys.path.insert(0, "/opt/trn_rl_repo")
    import concourse.bass as bass

import concourse.mybir as mybir
import concourse.tile as tile
from concourse import bacc
from concourse.bass import ts, ds
from concourse.bass_utils import run_bass_kernel_spmd

P = 128
D, H, E, TOPK = 1024, 4096, 8, 2
KD = D // P
MH = H // P
CT = 512

BF16 = ml_dtypes.bfloat16

_NC_CACHE: dict = {}


def _seg_chunks(base, seg, slot):
    """Column pieces of <=CT: near-equal large pieces plus a ~CT//2 tail
    piece (a short final piece shortens the end-of-kernel drain). Large
    pieces keep each matmul well past its LDWEIGHTS shadow."""
    tail = min(CT // 2, seg)
    head = seg - tail
    n = -(-head // CT) if head else 0
    out = []
    off = 0
    if n:
        per, rem = divmod(head, n)
        for i in range(n):
            cs = per + (1 if i < rem else 0)
            out.append((base + off, cs, slot))
            off += cs
    if tail:
        out.append((base + off, tail, slot))
        off += tail
    assert off == seg
    return out


def _build(SA: int, SB: int, repeat: int = 1):
    C = SA + SB
    nc = bacc.Bacc()
    AF = mybir.ActivationFunctionType
    xT = nc.dram_tensor("xT", [P, KD, C], mybir.dt.bfloat16, kind="ExternalInput")
    # w1 pre-tiled per m-block for streaming: [slot, m, p, k, col]
    w1 = nc.dram_tensor("w1", [2, MH, P, KD, P], mybir.dt.bfloat16, kind="ExternalInput")
    # w2 pre-tiled for stationary use in layer 2: [p(h), slot, m(h-blk), db, j]
    # = W2[m*P+p, db*P+j]
    w2 = nc.dram_tensor("w2", [P, 2, MH, KD, P], mybir.dt.bfloat16, kind="ExternalInput")
    b1 = nc.dram_tensor("b1", [P, 2, MH], mybir.dt.float32, kind="ExternalInput")
    # y transposed: [db, p(d), tok] — host untransposes; bf16 halves the
    # store traffic (combine happens in fp32 on host)
    y = nc.dram_tensor("y", [KD, P, C], mybir.dt.bfloat16, kind="ExternalOutput")

    # slot descriptors: (base, seg, slot); per-slot column sub-pieces <=CT.
    # Slot 0 processes its short tail piece FIRST (fast kernel start: small
    # first x transfer + short first m-blocks); slot 1 keeps it LAST
    # (short end-of-kernel drain).
    slots = [(0, SA, 0), (SA, SB, 1)]
    subs = {s: _seg_chunks(0, seg, s) for (base, seg, s) in slots}
    subs[0] = [subs[0][-1]] + subs[0][:-1]
    SMAX = max(SA, SB)

    with tile.TileContext(nc) as tc:
        with (
            tc.tile_pool(name="w2pool", bufs=1) as w2pool,
            tc.tile_pool(name="w1pool", bufs=4) as w1pool,
            tc.tile_pool(name="bpool", bufs=1) as bpool,
            tc.tile_pool(name="xpool", bufs=2) as xpool,
            tc.tile_pool(name="hpool", bufs=1) as hpool,
            tc.tile_pool(name="ypool", bufs=2) as ypool,
            tc.tile_pool(name="ps1", bufs=2, space="PSUM") as ps1_pool,
            tc.tile_pool(name="ps2", bufs=1, space="PSUM") as ps2_pool,
        ):
            # single W2 bank, reloaded per slot (whole-slot L1 hides the load)
            w2_sb = w2pool.tile([P, MH, KD, P], mybir.dt.bfloat16)
            b1_sb = bpool.tile([P, 2, MH], mybir.dt.float32)

            # warmup: keep PE busy during the initial DMA wait so the HAM
            # clock gate opens before the first real matmul (zeroed operands)
            warm_sb = bpool.tile([P, P], mybir.dt.bfloat16, name="warm")
            nc.vector.memset(warm_sb[:], 0.0)
            for g in range(2):
                wps = ps1_pool.tile([P, CT], mybir.dt.float32, name="ps")
                for i in range(8):
                    nc.tensor.matmul(
                        wps[:, :P],
                        warm_sb[:],
                        warm_sb[:],
                        start=(i == 0),
                        stop=(i == 7),
                    )

            def _w1_fetch(slot, m):
                blk = w1pool.tile([P, KD, P], mybir.dt.bfloat16)
                nc.sync.dma_start(out=blk[:], in_=w1[slot, m])
                return blk

            def _x_fetch(base, seg, fine=False):
                xt = xpool.tile([P, KD, SMAX], mybir.dt.bfloat16, name="x_sb")
                if fine:
                    # kernel start: short piece 0 as one descriptor, then
                    # w1 m=1..3 (piece-outer needs them early), then the
                    # big pieces in k-halves so compute tracks the stream
                    po, pl, _ = subs[0][0]
                    nc.sync.dma_start(
                        out=xt[:, :, ds(po, pl)],
                        in_=xT[:, :, ds(base + po, pl)],
                    )
                    while len(w1_pre) < 4:
                        w1_pre.append(_w1_fetch(0, len(w1_pre)))
                    nc.sync.dma_start(out=b1_sb[:], in_=b1[:])
                    for po, pl, _ in subs[0][1:]:
                        for q in range(2):
                            nc.sync.dma_start(
                                out=xt[:, ts(q, KD // 2), ds(po, pl)],
                                in_=xT[:, ts(q, KD // 2), ds(base + po, pl)],
                            )
                else:
                    for q in range(2):
                        nc.sync.dma_start(
                            out=xt[:, ts(q, KD // 2), :seg],
                            in_=xT[:, ts(q, KD // 2), ds(base, seg)],
                        )
                return xt

            # critical path first: slot-0's first w1 block, then its tokens
            w1_pre = [_w1_fetch(0, 0)]
            x_sb = _x_fetch(0, SA, fine=True)

            sched = [slots[i % 2] for i in range(2 * repeat)]
            DBG = 4  # db-group size for layer-2 psum accumulation

            for ci, (base, seg, slot) in enumerate(sched):
                pieces = subs[slot]
                h_sb = hpool.tile([P, MH, SMAX], mybir.dt.bfloat16)
                # at kernel start, run the first 4 m-blocks piece-outer so
                # layer-1 chews on piece 0 while pieces 1+ are still landing
                if ci == 0:
                    mp_order = [
                        (m, pc) for pc in range(len(pieces)) for m in range(4)
                    ] + [(m, pc) for m in range(4, MH) for pc in range(len(pieces))]
                else:
                    mp_order = [
                        (m, pc) for m in range(MH) for pc in range(len(pieces))
                    ]
                w1_blks = {}
                for m, pc in mp_order:
                    if pc == 0:
                        if m < len(w1_pre):
                            w1_blks[m] = w1_pre[m]
                        else:
                            w1_blks[m] = _w1_fetch(slot, m)
                        # this slot's W2 streams behind its own w1 blocks
                        # (k-progressive: L2 consumes slice s from k=8s)
                        if m in (12, 17, 22, 27):
                            s = (12, 17, 22, 27).index(m)
                            nc.sync.dma_start(
                                out=w2_sb[:, ts(s, MH // 4)],
                                in_=w2[:, slot, ts(s, MH // 4)],
                            )
                    w1_blk = w1_blks[m]
                    po, pl, _ = pieces[pc]
                    ps = ps1_pool.tile([P, CT], mybir.dt.float32)
                    for k in range(KD):
                        nc.tensor.matmul(
                            ps[:, :pl],
                            w1_blk[:, k, :],
                            x_sb[:, k, ds(po, pl)],
                            start=(k == 0),
                            stop=(k == KD - 1),
                        )
                    nc.scalar.activation(
                        h_sb[:, m, ds(po, pl)], ps[:, :pl], AF.Relu,
                        bias=b1_sb[:, slot, m : m + 1],
                    )
                # prefetch next slot's tokens + first w1 blocks so its
                # layer-1 starts without waiting behind this slot's L2
                if ci + 1 < len(sched):
                    nbase, nseg, nslot = sched[ci + 1]
                    x_sb = _x_fetch(nbase, nseg)
                    w1_pre = [_w1_fetch(nslot, 0), _w1_fetch(nslot, 1)]
                for dbg in range(KD // DBG):
                    for po, pl, _ in pieces:
                        pss = []
                        for j in range(DBG):
                            # j=0,1 double-buffered: piece p+1's first
                            # matmuls never wait on piece p's drains
                            ps2t = ps2_pool.tile(
                                [P, CT],
                                mybir.dt.float32,
                                name=f"ps2t{j}",
                                bufs=2 if j < 2 else 1,
                            )
                            pss.append(ps2t)
                        for k in range(MH):
                            for j in range(DBG):
                                nc.tensor.matmul(
                                    pss[j][:, :pl],
                                    w2_sb[:, k, dbg * DBG + j, :],
                                    h_sb[:, k, ds(po, pl)],
                                    start=(k == 0),
                                    stop=(k == MH - 1),
                                )
                        y_sb = ypool.tile([P, DBG, CT], mybir.dt.bfloat16)
                        last = (
                            ci == len(sched) - 1
                            and dbg == KD // DBG - 1
                            and po == pieces[-1][0]
                        )
                        # drain j=3 first: its matmul finishes last but its
                        # single-buffered bank is needed soonest by the
                        # next piece's k=0 accumulation group
                        for j in (3, 2, 1, 0):
                            # alternate DVE/ScalarE so drains run two-wide
                            if j % 2 == 0:
                                nc.vector.tensor_copy(
                                    y_sb[:, j, :pl], pss[j][:, :pl]
                                )
                            else:
                                nc.scalar.copy(y_sb[:, j, :pl], pss[j][:, :pl])
                            if last and j == 2:
                                # kernel tail: ship the done half early
                                nc.sync.dma_start(
                                    out=y[
                                        ds(dbg * DBG + 2, 2), :, ds(base + po, pl)
                                    ].rearrange("d p c -> p d c"),
                                    in_=y_sb[:, 2:, :pl],
                                )
                        if last:
                            nc.sync.dma_start(
                                out=y[
                                    ds(dbg * DBG, 2), :, ds(base + po, pl)
                                ].rearrange("d p c -> p d c"),
                                in_=y_sb[:, :2, :pl],
                            )
                        else:
                            # one descriptor per (db-group, piece)
                            nc.sync.dma_start(
                                out=y[
                                    ds(dbg * DBG, DBG), :, ds(base + po, pl)
                                ].rearrange("d p c -> p d c"),
                                in_=y_sb[:, :, :pl],
                            )
    nc.compile()
    return nc


def _route(x, Wr, br):
    xf = np.ascontiguousarray(np.asarray(x, np.float32).reshape(-1, D))
    T = xf.shape[0]
    logits = xf @ np.asarray(Wr, np.float32) + np.asarray(br, np.float32)
    top_i = np.argsort(-logits, axis=-1, kind="stable")[:, :TOPK]
    top_v = np.take_along_axis(logits, top_i, axis=-1)
    ex = np.exp(top_v - top_v.max(-1, keepdims=True))
    top_p = ex / ex.sum(-1, keepdims=True)

    ei = top_i.reshape(-1)
    perm = np.argsort(ei, kind="stable")
    counts = np.bincount(ei, minlength=E)
    tok = perm // TOPK
    p_sorted = top_p.reshape(-1)[perm]
    return xf, T, counts, perm, tok, p_sorted


def _plan(counts):
    """Pair experts (largest with smallest); split each expert's tokens in
    half across the pair's two cores. Returns uniform (unpadded) SA/SB and
    per-core (expertA, pieceA_start, pieceA_len, expertB, pieceB_start,
    pieceB_len)."""
    order = np.argsort(-counts, kind="stable")
    pairs = [(int(order[i]), int(order[E - 1 - i])) for i in range(E // 2)]

    SA = max(P, max(-(-int(counts[a]) // 2) for a, _ in pairs))
    SB = max(P, max(-(-int(counts[b]) // 2) for _, b in pairs))
    cores = []
    for a, b in pairs:
        a1 = int(-(-counts[a] // 2))
        b1 = int(-(-counts[b] // 2))
        cores.append((a, 0, a1, b, 0, b1))
        cores.append((a, a1, int(counts[a]) - a1, b, b1, int(counts[b]) - b1))
    return SA, SB, cores


def _prepare_in_maps(xf, counts, tok, W1, b1, W2, SA, SB, cores):
    W1 = np.asarray(W1, np.float32)
    W2 = np.asarray(W2, np.float32)
    b1 = np.asarray(b1, np.float32)
    offs = np.concatenate([[0], np.cumsum(counts)])
    C = SA + SB
    # per-expert prepped weights (shared between the pair's two cores)
    w1t = {}
    w2t = {}
    b1t = {}
    for e in set(int(c[0]) for c in cores) | set(int(c[3]) for c in cores):
        # [m, p, k, col] = W1[k*P+p, m*P+col]
        w1t[e] = np.ascontiguousarray(
            W1[e].reshape(KD, P, MH, P).transpose(2, 1, 0, 3).astype(BF16)
        )
        # [p(h), m, db, j] = W2[m*P+p, db*P+j]
        w2t[e] = np.ascontiguousarray(
            W2[e].reshape(MH, P, KD, P).transpose(1, 0, 2, 3).astype(BF16)
        )
        b1t[e] = np.ascontiguousarray(b1[e].reshape(MH, P).T)
    in_maps = []
    for a, sa, la, b, sb, lb in cores:
        xe = np.zeros((C, D), np.float32)
        xe[:la] = xf[tok[offs[a] + sa : offs[a] + sa + la]]
        xe[SA : SA + lb] = xf[tok[offs[b] + sb : offs[b] + sb + lb]]
        xT = np.ascontiguousarray(xe.reshape(C, KD, P).transpose(2, 1, 0)).astype(BF16)
        in_maps.append(
            {
                "xT": xT,
                "w1": np.stack([w1t[a], w1t[b]]),
                "w2": np.ascontiguousarray(
                    np.stack([w2t[a], w2t[b]], axis=1)
                ),
                "b1": np.ascontiguousarray(np.stack([b1t[a], b1t[b]], axis=1)),
            }
        )
    return in_maps


def _combine(results, counts, perm, p_sorted, b2, T, SA, SB, cores, out_shape):
    b2 = np.asarray(b2, np.float32)
    offs = np.concatenate([[0], np.cumsum(counts)])
    y_sorted = np.empty((int(counts.sum()), D), np.float32)
    for core, (a, sa, la, b, sb, lb) in enumerate(cores):
        # y is [db, p, C] with output dim d = db*P + p → [C, D]
        ye = results[core]["y"].reshape(D, -1).T.astype(np.float32)
        if la:
            y_sorted[offs[a] + sa : offs[a] + sa + la] = ye[:la] + b2[a]
        if lb:
            y_sorted[offs[b] + sb : offs[b] + sb + lb] = ye[SA : SA + lb] + b2[b]
    contrib = np.empty((T * TOPK, D), np.float32)
    contrib[perm] = y_sorted * p_sorted[:, None]
    return contrib.reshape(T, TOPK, D).sum(1).reshape(out_shape)


def _run(x, Wr, br, W1, b1, W2, b2, repeat: int = 1, timings: dict | None = None):
    import time

    xf, T, counts, perm, tok, p_sorted = _route(x, Wr, br)
    SA, SB, cores = _plan(counts)
    in_maps = _prepare_in_maps(xf, counts, tok, W1, b1, W2, SA, SB, cores)

    key = (SA, SB, repeat)
    if key not in _NC_CACHE:
        _NC_CACHE[key] = _build(SA, SB, repeat)
    nc = _NC_CACHE[key]

    t0 = time.time()
    res = run_bass_kernel_spmd(nc, in_maps, core_ids=list(range(E)))
    t1 = time.time()
    if timings is not None:
        timings["run_wall"] = t1 - t0
    return _combine(
        res.results, counts, perm, p_sorted, b2, T, SA, SB, cores,
        np.asarray(x).shape,
    )


def kernel(x, Wr, br, W1, b1, W2, b2):
    return _run(x, Wr, br, W1, b1, W2, b2).astype(np.float32)


# revision 47
# speedup vs baseline: 1.1786x; 1.1786x over previous
"""MoE layer (8 experts, top-2) on 8 Trainium2 NeuronCores — paired-expert load-balanced expert parallelism.

Experts are paired (largest with smallest by token count); each pair is
hosted by two cores, each core computing half of both experts' tokens.
Capacity per core C = SA + SB with UNPADDED segment sizes (no
128-alignment): both layers run column-exact matmuls, so padding waste
vs 128-rounding (~3%) is eliminated.

Device kernel (per slot, whole-segment loops):
  - x and h for the whole segment resident in SBUF; W1 streamed ONCE
    per slot per iteration (m-outer loop) — 42MB/core/iter total DMA.
  - Layer 1: out[h-block, tok] with W1 128x128 stationary blocks,
    tokens moving in <=512-column pieces.
  - Layer 2 swapped: out[d-block, tok] = W2-block stationary, h tokens
    moving — cost scales with exact token count, not 128-tiles. Output
    y is [d, tok] transposed, bf16; host untransposes and combines.
  - Single W2 SBUF bank (64KB/partition), reloaded per slot behind the
    slot's layer-1 (k-progressive slices feed layer-2's k-outer loop).
  - PSUM: 2 banks layer-1 double-buffer + 4-6 banks for 4 concurrent
    db-group accumulators; drains alternate DVE/ScalarE two-wide into a
    shared staging tile, one store descriptor per (db-group, piece).
"""

import numpy as np
import ml_dtypes

try:
    import concourse.bass as bass
except ImportError:
    import sys

    sys.path.insert(0, "/opt/trn_rl_repo")
    import concourse.bass as bass

import concourse.mybir as mybir
import concourse.tile as tile
from concourse import bacc
from concourse.bass import ts, ds
from concourse.bass_utils import run_bass_kernel_spmd

P = 128
D, H, E, TOPK = 1024, 4096, 8, 2
KD = D // P
MH = H // P
CT = 512

BF16 = ml_dtypes.bfloat16

_NC_CACHE: dict = {}


def _seg_chunks(base, seg, slot):
    """Column pieces of <=CT: near-equal large pieces plus a ~CT//2 tail
    piece (a short final piece shortens the end-of-kernel drain). Large
    pieces keep each matmul well past its LDWEIGHTS shadow."""
    tail = min(CT // 2, seg)
    head = seg - tail
    n = -(-head // CT) if head else 0
    out = []
    off = 0
    if n:
        per, rem = divmod(head, n)
        for i in range(n):
            cs = per + (1 if i < rem else 0)
            out.append((base + off, cs, slot))
            off += cs
    if tail:
        out.append((base + off, tail, slot))
        off += tail
    assert off == seg
    return out


def _build(SA: int, SB: int, repeat: int = 1):
    C = SA + SB
    nc = bacc.Bacc()
    AF = mybir.ActivationFunctionType
    xT = nc.dram_tensor("xT", [P, KD, C], mybir.dt.bfloat16, kind="ExternalInput")
    # w1 pre-tiled per m-block for streaming: [slot, m, p, k, col]
    w1 = nc.dram_tensor("w1", [2, MH, P, KD, P], mybir.dt.bfloat16, kind="ExternalInput")
    # w2 pre-tiled for stationary use in layer 2: [p(h), slot, m(h-blk), db, j]
    # = W2[m*P+p, db*P+j]
    w2 = nc.dram_tensor("w2", [P, 2, MH, KD, P], mybir.dt.bfloat16, kind="ExternalInput")
    b1 = nc.dram_tensor("b1", [P, 2, MH], mybir.dt.float32, kind="ExternalInput")
    # y transposed: [db, p(d), tok] — host untransposes; bf16 halves the
    # store traffic (combine happens in fp32 on host)
    y = nc.dram_tensor("y", [KD, P, C], mybir.dt.bfloat16, kind="ExternalOutput")

    # slot descriptors: (base, seg, slot); per-slot column sub-pieces <=CT.
    # Slot 0 processes its short tail piece FIRST (fast kernel start: small
    # first x transfer + short first m-blocks); slot 1 keeps it LAST
    # (short end-of-kernel drain).
    slots = [(0, SA, 0), (SA, SB, 1)]
    subs = {s: _seg_chunks(0, seg, s) for (base, seg, s) in slots}
    subs[0] = [subs[0][-1]] + subs[0][:-1]
    SMAX = max(SA, SB)

    with tile.TileContext(nc) as tc:
        with (
            tc.tile_pool(name="w2pool", bufs=1) as w2pool,
            tc.tile_pool(name="w1pool", bufs=4) as w1pool,
            tc.tile_pool(name="bpool", bufs=1) as bpool,
            tc.tile_pool(name="xpool", bufs=2) as xpool,
            tc.tile_pool(name="hpool", bufs=1) as hpool,
            tc.tile_pool(name="ypool", bufs=2) as ypool,
            tc.tile_pool(name="ps1", bufs=2, space="PSUM") as ps1_pool,
            tc.tile_pool(name="ps2", bufs=1, space="PSUM") as ps2_pool,
        ):
            # single W2 bank, reloaded per slot (whole-slot L1 hides the load)
            w2_sb = w2pool.tile([P, MH, KD, P], mybir.dt.bfloat16)
            b1_sb = bpool.tile([P, 2, MH], mybir.dt.float32)

            # warmup: keep PE busy during the initial DMA wait so the HAM
            # clock gate opens before the first real matmul (zeroed operands)
            warm_sb = bpool.tile([P, P], mybir.dt.bfloat16, name="warm")
            nc.vector.memset(warm_sb[:], 0.0)
            # preload the Relu activation table during the DMA wait so the
            # first real layer-1 activation doesn't pay the ~1.3us load
            act_warm = bpool.tile([P, 1], mybir.dt.bfloat16, name="actwarm")
            nc.scalar.activation(act_warm[:], warm_sb[:, :1], AF.Relu)
            for g in range(2):
                wps = ps1_pool.tile([P, CT], mybir.dt.float32, name="ps")
                for i in range(8):
                    nc.tensor.matmul(
                        wps[:, :P],
                        warm_sb[:],
                        warm_sb[:],
                        start=(i == 0),
                        stop=(i == 7),
                    )

            def _w1_fetch(slot, m):
                blk = w1pool.tile([P, KD, P], mybir.dt.bfloat16)
                nc.sync.dma_start(out=blk[:], in_=w1[slot, m])
                return blk

            def _x_fetch(base, seg, fine=False):
                xt = xpool.tile([P, KD, SMAX], mybir.dt.bfloat16, name="x_sb")
                if fine:
                    # kernel start: short piece 0 as one descriptor, then
                    # w1 m=1..3 (piece-outer needs them early), then the
                    # big pieces in k-halves so compute tracks the stream
                    po, pl, _ = subs[0][0]
                    nc.sync.dma_start(
                        out=xt[:, :, ds(po, pl)],
                        in_=xT[:, :, ds(base + po, pl)],
                    )
                    while len(w1_pre) < 4:
                        w1_pre.append(_w1_fetch(0, len(w1_pre)))
                    nc.sync.dma_start(out=b1_sb[:], in_=b1[:])
                    for po, pl, _ in subs[0][1:]:
                        for q in range(2):
                            nc.sync.dma_start(
                                out=xt[:, ts(q, KD // 2), ds(po, pl)],
                                in_=xT[:, ts(q, KD // 2), ds(base + po, pl)],
                            )
                else:
                    for q in range(2):
                        nc.sync.dma_start(
                            out=xt[:, ts(q, KD // 2), :seg],
                            in_=xT[:, ts(q, KD // 2), ds(base, seg)],
                        )
                return xt

            # critical path first: slot-0's first w1 block, then its tokens
            w1_pre = [_w1_fetch(0, 0)]
            x_sb = _x_fetch(0, SA, fine=True)

            sched = [slots[i % 2] for i in range(2 * repeat)]
            DBG = 4  # db-group size for layer-2 psum accumulation

            for ci, (base, seg, slot) in enumerate(sched):
                pieces = subs[slot]
                h_sb = hpool.tile([P, MH, SMAX], mybir.dt.bfloat16)
                # at kernel start, run the first 4 m-blocks piece-outer so
                # layer-1 chews on piece 0 while pieces 1+ are still landing
                if ci == 0:
                    mp_order = [
                        (m, pc) for pc in range(len(pieces)) for m in range(4)
                    ] + [(m, pc) for m in range(4, MH) for pc in range(len(pieces))]
                else:
                    mp_order = [
                        (m, pc) for m in range(MH) for pc in range(len(pieces))
                    ]
                w1_blks = {}
                for m, pc in mp_order:
                    if pc == 0:
                        if m < len(w1_pre):
                            w1_blks[m] = w1_pre[m]
                        else:
                            w1_blks[m] = _w1_fetch(slot, m)
                        # this slot's W2 streams behind its own w1 blocks
                        # (k-progressive: L2 consumes slice s from k=8s)
                        if m in (12, 17, 22, 27):
                            s = (12, 17, 22, 27).index(m)
                            nc.sync.dma_start(
                                out=w2_sb[:, ts(s, MH // 4)],
                                in_=w2[:, slot, ts(s, MH // 4)],
                            )
                    w1_blk = w1_blks[m]
                    po, pl, _ = pieces[pc]
                    ps = ps1_pool.tile([P, CT], mybir.dt.float32)
                    for k in range(KD):
                        nc.tensor.matmul(
                            ps[:, :pl],
                            w1_blk[:, k, :],
                            x_sb[:, k, ds(po, pl)],
                            start=(k == 0),
                            stop=(k == KD - 1),
                        )
                    nc.scalar.activation(
                        h_sb[:, m, ds(po, pl)], ps[:, :pl], AF.Relu,
                        bias=b1_sb[:, slot, m : m + 1],
                    )
                # prefetch next slot's tokens + first w1 blocks so its
                # layer-1 starts without waiting behind this slot's L2
                if ci + 1 < len(sched):
                    nbase, nseg, nslot = sched[ci + 1]
                    x_sb = _x_fetch(nbase, nseg)
                    w1_pre = [_w1_fetch(nslot, 0), _w1_fetch(nslot, 1)]
                for dbg in range(KD // DBG):
                    for po, pl, _ in pieces:
                        pss = []
                        for j in range(DBG):
                            # j=0,1 double-buffered: piece p+1's first
                            # matmuls never wait on piece p's drains
                            ps2t = ps2_pool.tile(
                                [P, CT],
                                mybir.dt.float32,
                                name=f"ps2t{j}",
                                bufs=2 if j < 2 else 1,
                            )
                            pss.append(ps2t)
                        for k in range(MH):
                            for j in range(DBG):
                                nc.tensor.matmul(
                                    pss[j][:, :pl],
                                    w2_sb[:, k, dbg * DBG + j, :],
                                    h_sb[:, k, ds(po, pl)],
                                    start=(k == 0),
                                    stop=(k == MH - 1),
                                )
                        y_sb = ypool.tile([P, DBG, CT], mybir.dt.bfloat16)
                        last = (
                            ci == len(sched) - 1
                            and dbg == KD // DBG - 1
                            and po == pieces[-1][0]
                        )
                        # drain j=3 first: its matmul finishes last but its
                        # single-buffered bank is needed soonest by the
                        # next piece's k=0 accumulation group
                        for j in (3, 2, 1, 0):
                            # alternate DVE/ScalarE so drains run two-wide
                            if j % 2 == 0:
                                nc.vector.tensor_copy(
                                    y_sb[:, j, :pl], pss[j][:, :pl]
                                )
                            else:
                                nc.scalar.copy(y_sb[:, j, :pl], pss[j][:, :pl])
                            if last and j == 2:
                                # kernel tail: ship the done half early
                                nc.sync.dma_start(
                                    out=y[
                                        ds(dbg * DBG + 2, 2), :, ds(base + po, pl)
                                    ].rearrange("d p c -> p d c"),
                                    in_=y_sb[:, 2:, :pl],
                                )
                        if last:
                            nc.sync.dma_start(
                                out=y[
                                    ds(dbg * DBG, 2), :, ds(base + po, pl)
                                ].rearrange("d p c -> p d c"),
                                in_=y_sb[:, :2, :pl],
                            )
                        else:
                            # one descriptor per (db-group, piece)
                            nc.sync.dma_start(
                                out=y[
                                    ds(dbg * DBG, DBG), :, ds(base + po, pl)
                                ].rearrange("d p c -> p d c"),
                                in_=y_sb[:, :, :pl],
                            )
    nc.compile()
    return nc


def _route(x, Wr, br):
    xf = np.ascontiguousarray(np.asarray(x, np.float32).reshape(-1, D))
    T = xf.shape[0]
    logits = xf @ np.asarray(Wr, np.float32) + np.asarray(br, np.float32)
    top_i = np.argsort(-logits, axis=-1, kind="stable")[:, :TOPK]
    top_v = np.take_along_axis(logits, top_i, axis=-1)
    ex = np.exp(top_v - top_v.max(-1, keepdims=True))
    top_p = ex / ex.sum(-1, keepdims=True)

    ei = top_i.reshape(-1)
    perm = np.argsort(ei, kind="stable")
    counts = np.bincount(ei, minlength=E)
    tok = perm // TOPK
    p_sorted = top_p.reshape(-1)[perm]
    return xf, T, counts, perm, tok, p_sorted


def _plan(counts):
    """Pair experts (largest with smallest); split each expert's tokens in
    half across the pair's two cores. Returns uniform (unpadded) SA/SB and
    per-core (expertA, pieceA_start, pieceA_len, expertB, pieceB_start,
    pieceB_len)."""
    order = np.argsort(-counts, kind="stable")
    pairs = [(int(order[i]), int(order[E - 1 - i])) for i in range(E // 2)]

    SA = max(P, max(-(-int(counts[a]) // 2) for a, _ in pairs))
    SB = max(P, max(-(-int(counts[b]) // 2) for _, b in pairs))
    cores = []
    for a, b in pairs:
        a1 = int(-(-counts[a] // 2))
        b1 = int(-(-counts[b] // 2))
        cores.append((a, 0, a1, b, 0, b1))
        cores.append((a, a1, int(counts[a]) - a1, b, b1, int(counts[b]) - b1))
    return SA, SB, cores


def _prepare_in_maps(xf, counts, tok, W1, b1, W2, SA, SB, cores):
    W1 = np.asarray(W1, np.float32)
    W2 = np.asarray(W2, np.float32)
    b1 = np.asarray(b1, np.float32)
    offs = np.concatenate([[0], np.cumsum(counts)])
    C = SA + SB
    # per-expert prepped weights (shared between the pair's two cores)
    w1t = {}
    w2t = {}
    b1t = {}
    for e in set(int(c[0]) for c in cores) | set(int(c[3]) for c in cores):
        # [m, p, k, col] = W1[k*P+p, m*P+col]
        w1t[e] = np.ascontiguousarray(
            W1[e].reshape(KD, P, MH, P).transpose(2, 1, 0, 3).astype(BF16)
        )
        # [p(h), m, db, j] = W2[m*P+p, db*P+j]
        w2t[e] = np.ascontiguousarray(
            W2[e].reshape(MH, P, KD, P).transpose(1, 0, 2, 3).astype(BF16)
        )
        b1t[e] = np.ascontiguousarray(b1[e].reshape(MH, P).T)
    in_maps = []
    for a, sa, la, b, sb, lb in cores:
        xe = np.zeros((C, D), np.float32)
        xe[:la] = xf[tok[offs[a] + sa : offs[a] + sa + la]]
        xe[SA : SA + lb] = xf[tok[offs[b] + sb : offs[b] + sb + lb]]
        xT = np.ascontiguousarray(xe.reshape(C, KD, P).transpose(2, 1, 0)).astype(BF16)
        in_maps.append(
            {
                "xT": xT,
                "w1": np.stack([w1t[a], w1t[b]]),
                "w2": np.ascontiguousarray(
                    np.stack([w2t[a], w2t[b]], axis=1)
                ),
                "b1": np.ascontiguousarray(np.stack([b1t[a], b1t[b]], axis=1)),
            }
        )
    return in_maps


def _combine(results, counts, perm, p_sorted, b2, T, SA, SB, cores, out_shape):
    b2 = np.asarray(b2, np.float32)
    offs = np.concatenate([[0], np.cumsum(counts)])
    y_sorted = np.empty((int(counts.sum()), D), np.float32)
    for core, (a, sa, la, b, sb, lb) in enumerate(cores):
        # y is [db, p, C] with output dim d = db*P + p → [C, D]
        ye = results[core]["y"].reshape(D, -1).T.astype(np.float32)
        if la:
            y_sorted[offs[a] + sa : offs[a] + sa + la] = ye[:la] + b2[a]
        if lb:
            y_sorted[offs[b] + sb : offs[b] + sb + lb] = ye[SA : SA + lb] + b2[b]
    contrib = np.empty((T * TOPK, D), np.float32)
    contrib[perm] = y_sorted * p_sorted[:, None]
    return contrib.reshape(T, TOPK, D).sum(1).reshape(out_shape)


def _run(x, Wr, br, W1, b1, W2, b2, repeat: int = 1, timings: dict | None = None):
    import time

    xf, T, counts, perm, tok, p_sorted = _route(x, Wr, br)
    SA, SB, cores = _plan(counts)
    in_maps = _prepare_in_maps(xf, counts, tok, W1, b1, W2, SA, SB, cores)

    key = (SA, SB, repeat)
    if key not in _NC_CACHE:
        _NC_CACHE[key] = _build(SA, SB, repeat)
    nc = _NC_CACHE[key]

    t0 = time.time()
    res = run_bass_kernel_spmd(nc, in_maps, core_ids=list(range(E)))
    t1 = time.time()
    if timings is not None:
        timings["run_wall"] = t1 - t0
    return _combine(
        res.results, counts, perm, p_sorted, b2, T, SA, SB, cores,
        np.asarray(x).shape,
    )


def kernel(x, Wr, br, W1, b1, W2, b2):
    return _run(x, Wr, br, W1, b1, W2, b2).astype(np.float32)
